# revision 6
# baseline (speedup 1.0000x reference)
"""Trainium2 Bass kernel for nn_GATTNetConvHybrid (EdgeConv x2 + GAT x2 + cls).

8-core graph-parallel strategy:
  - nodes block-partitioned: core c owns rows [c*3750, (c+1)*3750)
  - edges assigned by dst, sorted by dst, tiled in 128-edge tiles grouped by
    128-node dst windows; scatter = one-hot matmul accumulating in PSUM
  - halo exchange: AllGather of per-node linear maps (B-arrays / h-arrays);
    per-edge rows fetched with indirect DMA from the gathered buffer
  - EdgeConv algebra folded: BN into weights, fc1/short split by endpoint
    linearity, fc3 pulled out of the segment sum
  - GAT softmax uses raw exp with a constant shift (logits bounded; verified)
"""
import math
import numpy as np

import concourse.bass as bass
import concourse.mybir as mybir
import concourse.tile as tile
from concourse.masks import make_identity
from concourse.vector_clock import ScopedClock

F32 = mybir.dt.float32
I32 = mybir.dt.int32
AF = mybir.ActivationFunctionType
ALU = mybir.AluOpType

P = 128
EPS = 1e-5
HEADS = 4
SHIFT1 = 20.0  # constant softmax shift, gat1 (logits observed <= ~52)
SHIFT2 = 10.0  # gat2 (logits <= ~28)

# ---------------------------------------------------------------- tile patch
# This container's walrus accepts only ONE sync-wait condition per
# instruction; split the TileContext tail drain and any multi-wait
# instruction into chains carrying one wait each.


def _drain_and_barrier_split(self, tick_clock, wait_clock):
    nc = self.nc
    drain_inst = nc.sync.drain()
    wait_clock.add_sem_waits(
        drain_inst.ins, ScopedClock({None: tick_clock.global_clock})
    )
    si = drain_inst.ins.sync_info
    waits = list(si.on_wait or [])
    if len(waits) > 1:
        si.on_wait = waits[:1]
        drain_inst.ins.sync_info = si
        for i in range(1, len(waits)):
            extra = nc.sync.drain()
            extra.ins.sync_info = mybir.SyncInfo(on_wait=waits[i:i + 1], on_update=[])
    nc.all_engine_barrier()
    assert self.sems is not None
    popped = nc._tile_sem_poison_stack.pop()
    assert popped is self._sem_poison
    nc.clear_and_free_semaphores(list(self.sems.allocated().values()))
    nc.all_engine_barrier()


tile.TileContext._drain_and_barrier = _drain_and_barrier_split
_noop_ctr = [0]


def split_sync_waits(nc):
    n_split = 0
    for fn in nc.m.functions:
        for bb in fn.blocks:
            il = bb.instructions
            new_insts = []
            changed = False
            for inst in il:
                si = inst.sync_info
                waits = list(si.on_wait) if si is not None and si.on_wait else []
                if len(waits) > 1:
                    n_split += 1
                    changed = True
                    for i in range(1, len(waits)):
                        _noop_ctr[0] += 1
                        nop = mybir.InstNoOp(
                            name=f"I-waitsplit-{_noop_ctr[0]}", ins=[], outs=[])
                        nop.engine = inst.engine
                        nop.sync_info = mybir.SyncInfo(
                            on_wait=waits[i:i + 1], on_update=[])
                        new_insts.append(nop)
                        nc.register_instruction(nop, overwrite=True)
                    si.on_wait = waits[:1]
                    inst.sync_info = si
                new_insts.append(inst)
            if changed:
                il[:] = new_insts
    return n_split


# ------------------------------------------------------------- param folding

def _f32(a):
    return np.asarray(a, dtype=np.float32)


def fold_ec(p):
    W1, b1 = _f32(p["fc1"]["w"]), _f32(p["fc1"]["b"])
    F = W1.shape[0] // 2
    s1 = _f32(p["bn1"]["g"]) / np.sqrt(_f32(p["bn1"]["v"]) + EPS)
    c1 = (b1 - _f32(p["bn1"]["m"])) * s1 + _f32(p["bn1"]["b"])
    W1a, W1b = W1[:F] * s1, W1[F:] * s1
    s2 = _f32(p["bn2"]["g"]) / np.sqrt(_f32(p["bn2"]["v"]) + EPS)
    c2 = (_f32(p["fc2"]["b"]) - _f32(p["bn2"]["m"])) * s2 + _f32(p["bn2"]["b"])
    W2p = _f32(p["fc2"]["w"]) * s2
    W3, b3 = _f32(p["fc3"]["w"]), _f32(p["fc3"]["b"])
    Ws, bs = _f32(p["short"]["w"]), _f32(p["short"]["b"])
    return dict(WA=np.ascontiguousarray(W1a - W1b), WB=np.ascontiguousarray(W1b),
                c1=c1, W2p=W2p, c2=c2, W3=W3,
                WSa=np.ascontiguousarray(Ws[:F] - Ws[F:]),
                WSb=np.ascontiguousarray(Ws[F:]), cS=b3 + bs)


# ---------------------------------------------------------- graph structures

def build_graph(edge_index, n_nodes, n_cores):
    src = np.asarray(edge_index[0], dtype=np.int64)
    dst = np.asarray(edge_index[1], dtype=np.int64)
    n_own = n_nodes // n_cores
    nw = math.ceil(n_own / P)
    order = np.argsort(dst, kind="stable")
    ssrc, sdst = src[order], dst[order]
    core_of = sdst // n_own
    win_of = (sdst % n_own) // P
    counts = np.zeros((n_cores, nw), dtype=np.int64)
    np.add.at(counts, (core_of, win_of), 1)
    tw = np.maximum(1, np.ceil(counts.max(axis=0) / P).astype(np.int64))
    t_all = int(tw.sum())
    src_idx = np.zeros((n_cores, P, t_all), dtype=np.int32)
    dstloc = np.full((n_cores, P, t_all), -1.0, dtype=np.float32)
    for c in range(n_cores):
        m = core_of == c
        es, ed = ssrc[m], sdst[m]
        wins = win_of[m]
        t0 = 0
        for w in range(nw):
            wm = wins == w
            k = int(wm.sum())
            cap = int(tw[w]) * P
            assert k <= cap
            col = np.zeros(cap, dtype=np.int32)
            dl = np.full(cap, -1.0, dtype=np.float32)
            col[:k] = es[wm]
            dl[:k] = (ed[wm] - c * n_own - w * P).astype(np.float32)
            src_idx[c, :, t0:t0 + int(tw[w])] = col.reshape(int(tw[w]), P).T
            dstloc[c, :, t0:t0 + int(tw[w])] = dl.reshape(int(tw[w]), P).T
            t0 += int(tw[w])
    deg = np.bincount(dst, minlength=n_nodes).astype(np.float32)
    degw = np.zeros((n_cores, P, nw), dtype=np.float32)
    invw = np.ones((n_cores, P, nw), dtype=np.float32)
    for c in range(n_cores):
        d = deg[c * n_own:(c + 1) * n_own]
        d = np.pad(d, (0, nw * P - len(d)))
        degw[c] = d.reshape(nw, P).T
        invw[c] = (1.0 / np.maximum(d, 1.0)).reshape(nw, P).T
    return dict(tw=[int(t) for t in tw], t_all=t_all, nw=nw, n_own=n_own,
                src_idx=src_idx, dstloc=dstloc, degw=degw, invw=invw)


def rep(v):
    v = _f32(v).reshape(1, -1)
    return np.ascontiguousarray(np.repeat(v, P, axis=0))


# ------------------------------------------------------------- the program

def build_program(g, n_cores):
    nc = bass.Bass("TRN2", target_bir_lowering=False, debug=False,
                   num_devices=n_cores)
    nw, n_own, tw = g["nw"], g["n_own"], g["tw"]
    nvalid = n_own - (nw - 1) * P
    N = n_own * n_cores
    rg = [list(range(n_cores))]

    def inp(name, shape, dtype=F32):
        return nc.dram_tensor(name, list(shape), dtype, kind="ExternalInput").ap()

    x_in = inp("x_own", (n_own, 128))
    srcix = inp("src_idx", (P, g["t_all"]), I32)
    dloc = inp("dstloc", (P, g["t_all"]))
    deg_in = inp("degw", (P, nw))
    inv_in = inp("invw", (P, nw))
    ecw = {}
    for l in (1, 2):
        for n in ("WA", "WB", "WSa", "WSb", "W2p", "W3"):
            ecw[(l, n)] = inp(f"ec{l}_{n}", (128, 128))
        for n in ("c1", "c2", "cS"):
            ecw[(l, n)] = inp(f"ec{l}_{n}r", (P, 128))
    wg1 = inp("wg1", (256, 512))
    wg2 = inp("wg2", (512, 512))
    asr1 = inp("asrc1r", (P, 512))
    adr1 = inp("adst1r", (P, 512))
    asr2 = inp("asrc2r", (P, 512))
    adr2 = inp("adst2r", (P, 512))
    bg1 = inp("bg1r", (P, 512))
    bg2 = inp("bg2r", (P, 128))
    wcls = inp("wcls", (128, 40))
    bcls = inp("bclsr", (P, 40))

    out_t = nc.dram_tensor("out", [n_own, 40], F32, kind="ExternalOutput").ap()

    cc_in1 = nc.dram_tensor("cc_in1", [n_own, 256], F32)
    cc_out1 = nc.dram_tensor("cc_out1", [N, 256], F32, addr_space="Shared")
    cc_in2 = nc.dram_tensor("cc_in2", [n_own, 256], F32)
    cc_out2 = nc.dram_tensor("cc_out2", [N, 256], F32, addr_space="Shared")
    cc_ing1 = nc.dram_tensor("cc_ing1", [n_own, 516], F32)
    cc_outg1 = nc.dram_tensor("cc_outg1", [N, 516], F32, addr_space="Shared")
    cc_ing2 = nc.dram_tensor("cc_ing2", [n_own, 516], F32)
    cc_outg2 = nc.dram_tensor("cc_outg2", [N, 516], F32, addr_space="Shared")
    g1_dram = nc.dram_tensor("g1_dram", [n_own, 512], F32)

    with tile.TileContext(nc) as tc:
        import contextlib
        with contextlib.ExitStack() as ctx:
            resid = ctx.enter_context(tc.tile_pool(name="resid", bufs=1))
            work = ctx.enter_context(tc.tile_pool(name="work", bufs=2))
            gpool = ctx.enter_context(tc.tile_pool(name="gath", bufs=4))
            psum = ctx.enter_context(tc.tile_pool(name="psum", bufs=2, space="PSUM"))

            # ------------ resident constants
            ident = resid.tile([P, P], F32, tag="ident")
            make_identity(nc, ident[:])
            iota_i = resid.tile([P, P], I32, tag="iota_i")
            nc.gpsimd.iota(iota_i[:], pattern=[[1, P]], base=0, channel_multiplier=0)
            iota_row = resid.tile([P, P], F32, tag="iota_row")
            nc.vector.tensor_copy(out=iota_row[:], in_=iota_i[:])

            six = resid.tile([P, g["t_all"]], I32, tag="six")
            nc.sync.dma_start(out=six[:], in_=srcix[:, :])
            dlt = resid.tile([P, g["t_all"]], F32, tag="dlt")
            nc.sync.dma_start(out=dlt[:], in_=dloc[:, :])
            degt = resid.tile([P, nw], F32, tag="degt")
            nc.sync.dma_start(out=degt[:], in_=deg_in[:, :])
            invt = resid.tile([P, nw], F32, tag="invt")
            nc.sync.dma_start(out=invt[:], in_=inv_in[:, :])
            sh1t = resid.tile([P, 1], F32, tag="sh1t")
            nc.vector.memset(sh1t[:], -SHIFT1)
            sh2t = resid.tile([P, 1], F32, tag="sh2t")
            nc.vector.memset(sh2t[:], -SHIFT2)

            def load_w(name, ap, w):
                t = resid.tile([P, w], F32, tag=name)
                nc.sync.dma_start(out=t[:], in_=ap[:, :])
                return t
            ecs = {}
            for (l, n), ap in ecw.items():
                ecs[(l, n)] = load_w(f"ec{l}{n}", ap, 128)
            wg1t = resid.tile([P, 2, 512], F32, tag="wg1t")
            nc.sync.dma_start(out=wg1t[:], in_=wg1.rearrange("(o p) f -> p o f", p=P))
            wg2t = resid.tile([P, 4, 512], F32, tag="wg2t")
            nc.sync.dma_start(out=wg2t[:], in_=wg2.rearrange("(o p) f -> p o f", p=P))
            asr1t = load_w("asr1t", asr1, 512)
            adr1t = load_w("adr1t", adr1, 512)
            asr2t = load_w("asr2t", asr2, 512)
            adr2t = load_w("adr2t", adr2, 512)
            bg1t = load_w("bg1t", bg1, 512)
            bg2t = load_w("bg2t", bg2, 128)
            wclst = load_w("wclst", wcls, 40)
            bclst = load_w("bclst", bcls, 40)

            # x windows, node-major, zero-padded tail
            x_w = resid.tile([P, nw, 128], F32, tag="xB")
            for w in range(nw):
                rows = P if w < nw - 1 else nvalid
                if rows < P:
                    nc.vector.memset(x_w[:, w, :], 0.0)
                nc.sync.dma_start(out=x_w[:rows, w, :], in_=x_in[w * P:w * P + rows, :])

            # ---------------- helpers
            def transpose_to_sbuf(src_ap, dst_ap, relu=False):
                pt = psum.tile([P, P], F32, tag="tp")
                nc.tensor.transpose(pt[:], src_ap, ident[:])
                nc.scalar.activation(dst_ap, pt[:], AF.Relu if relu else AF.Copy)

            # ================= EdgeConv layers =================
            def ec_layer(l, xin_w, cc_in, cc_out, mean, xo_tag):
                Aw = resid.tile([P, nw, 128], F32, tag="Aw")
                Ash = resid.tile([P, nw, 128], F32, tag="Ash")
                for w in range(nw):
                    fmt = work.tile([P, P], F32, tag="fm")
                    transpose_to_sbuf(xin_w[:, w, :], fmt[:])
                    rows = P if w < nw - 1 else nvalid
                    for name, coff in (("WA", -1), ("WSa", -2), ("WB", 0),
                                       ("WSb", 128)):
                        po = psum.tile([P, 128], F32, tag="ep")
                        nc.tensor.matmul(po[:], lhsT=fmt[:], rhs=ecs[(l, name)][:],
                                         start=True, stop=True)
                        if coff == -1:
                            nc.vector.tensor_add(out=Aw[:, w, :], in0=po[:],
                                                 in1=ecs[(l, "c1")][:])
                        elif coff == -2:
                            nc.vector.tensor_add(out=Ash[:, w, :], in0=po[:],
                                                 in1=ecs[(l, "cS")][:])
                        else:
                            bt = work.tile([P, 128], F32, tag="bt")
                            nc.scalar.activation(bt[:], po[:], AF.Copy)
                            nc.sync.dma_start(
                                out=cc_in[w * P:w * P + rows, coff:coff + 128],
                                in_=bt[:rows, :])
                nc.gpsimd.collective_compute(
                    "AllGather", ALU.bypass, ins=[cc_in.ap()],
                    outs=[cc_out.ap()], replica_groups=rg)
                xo = resid.tile([P, nw, 128], F32, tag=xo_tag)
                t0 = 0
                for w in range(nw):
                    T = tw[w]
                    Mw = psum.tile([P, 128], F32, tag="acc1")
                    Sw = psum.tile([P, 128], F32, tag="acc2")
                    for t in range(T):
                        gt = t0 + t
                        gtl = gpool.tile([P, 256], F32, tag="g")
                        nc.gpsimd.indirect_dma_start(
                            out=gtl[:], out_offset=None, in_=cc_out[:, :],
                            in_offset=bass.IndirectOffsetOnAxis(
                                ap=six[:, gt:gt + 1], axis=0))
                        O = work.tile([P, P], F32, tag="O")
                        nc.vector.tensor_tensor(
                            out=O[:], in0=dlt[:, gt:gt + 1].to_broadcast([P, P]),
                            in1=iota_row[:], op=ALU.is_equal)
                        OT = work.tile([P, P], F32, tag="OT")
                        transpose_to_sbuf(O[:], OT[:])
                        e1p = psum.tile([P, P], F32, tag="ep")
                        nc.tensor.matmul(e1p[:], lhsT=gtl[:, 0:128], rhs=ident[:],
                                         is_transpose=True, start=True, stop=False,
                                         skip_group_check=True)
                        nc.tensor.matmul(e1p[:], lhsT=Aw[:, w, :], rhs=OT[:],
                                         start=False, stop=True,
                                         skip_group_check=True)
                        e1T = work.tile([P, P], F32, tag="e1T")
                        nc.scalar.activation(e1T[:], e1p[:], AF.Relu)
                        e2p = psum.tile([P, P], F32, tag="ep")
                        nc.tensor.matmul(e2p[:], lhsT=e1T[:],
                                         rhs=ecs[(l, "W2p")][:], start=True, stop=True)
                        e2 = work.tile([P, P], F32, tag="e2")
                        nc.vector.tensor_add(out=e2[:], in0=e2p[:],
                                             in1=ecs[(l, "c2")][:])
                        nc.scalar.activation(e2[:], e2[:], AF.Relu)
                        nc.tensor.matmul(Mw[:], lhsT=O[:], rhs=e2[:],
                                         start=(t == 0), stop=(t == T - 1),
                                         skip_group_check=True)
                        nc.tensor.matmul(Sw[:], lhsT=O[:], rhs=gtl[:, 128:256],
                                         start=(t == 0), stop=(t == T - 1),
                                         skip_group_check=True)
                    Mnm = work.tile([P, P], F32, tag="Mnm")
                    nc.scalar.activation(Mnm[:], Mw[:], AF.Copy)
                    Mfm = work.tile([P, P], F32, tag="Mfm")
                    transpose_to_sbuf(Mnm[:], Mfm[:])
                    po = psum.tile([P, 128], F32, tag="ep")
                    nc.tensor.matmul(po[:], lhsT=Mfm[:], rhs=ecs[(l, "W3")][:],
                                     start=True, stop=True)
                    t1 = work.tile([P, 128], F32, tag="t1")
                    nc.vector.tensor_scalar_mul(t1[:], Ash[:, w, :], degt[:, w:w + 1])
                    nc.vector.tensor_add(out=t1[:], in0=t1[:], in1=po[:])
                    nc.vector.tensor_add(out=t1[:], in0=t1[:], in1=Sw[:])
                    if mean:
                        nc.vector.tensor_scalar_mul(t1[:], t1[:], invt[:, w:w + 1])
                    nc.vector.scalar_tensor_tensor(
                        out=xo[:, w, :], in0=t1[:], scalar=0.01, in1=t1[:],
                        op0=ALU.mult, op1=ALU.max)
                    t0 += T
                return xo

            x1_w = ec_layer(1, x_w, cc_in1, cc_out1, mean=False, xo_tag="xC")
            x2_w = ec_layer(2, x1_w, cc_in2, cc_out2, mean=True, xo_tag="xB")

            # ================= GAT layers =================
            def gat_layer(l, chunk_fn, nfc, wgt, asrt, adrt, cc_in, cc_out,
                          shift_t, concat, bias_t, out_writer):
                as_w = resid.tile([P, nw, 4], F32, tag=f"as{l}")
                ad_w = resid.tile([P, nw, 4], F32, tag=f"ad{l}")
                for w in range(nw):
                    hp = psum.tile([P, 512], F32, tag="ep")
                    for fc in range(nfc):
                        fmt = work.tile([P, P], F32, tag="fm")
                        transpose_to_sbuf(chunk_fn(w, fc), fmt[:])
                        nc.tensor.matmul(hp[:], lhsT=fmt[:], rhs=wgt[:, fc, :],
                                         start=(fc == 0), stop=(fc == nfc - 1))
                    hw = work.tile([P, 512], F32, tag="hw")
                    nc.scalar.activation(hw[:], hp[:], AF.Copy)
                    tmp = work.tile([P, 512], F32, tag="tmp5")
                    nc.vector.tensor_mul(out=tmp[:], in0=hw[:], in1=asrt[:])
                    nc.vector.tensor_reduce(
                        out=as_w[:, w, :], in_=tmp[:].rearrange("p (h c) -> p h c", h=4),
                        axis=mybir.AxisListType.X, op=ALU.add)
                    nc.vector.tensor_mul(out=tmp[:], in0=hw[:], in1=adrt[:])
                    nc.vector.tensor_reduce(
                        out=ad_w[:, w, :], in_=tmp[:].rearrange("p (h c) -> p h c", h=4),
                        axis=mybir.AxisListType.X, op=ALU.add)
                    rows = P if w < nw - 1 else nvalid
                    nc.sync.dma_start(out=cc_in[w * P:w * P + rows, 0:512],
                                      in_=hw[:rows, :])
                    nc.sync.dma_start(out=cc_in[w * P:w * P + rows, 512:516],
                                      in_=as_w[:rows, w, :])
                nc.gpsimd.collective_compute(
                    "AllGather", ALU.bypass, ins=[cc_in.ap()],
                    outs=[cc_out.ap()], replica_groups=rg)
                t0 = 0
                for w in range(nw):
                    T = tw[w]
                    Mw = psum.tile([P, 512], F32, tag="acc1")
                    Dw = psum.tile([P, 4], F32, tag="acc2")
                    for t in range(T):
                        gt = t0 + t
                        G = gpool.tile([P, 516], F32, tag="G")
                        nc.gpsimd.indirect_dma_start(
                            out=G[:], out_offset=None, in_=cc_out[:, :],
                            in_offset=bass.IndirectOffsetOnAxis(
                                ap=six[:, gt:gt + 1], axis=0))
                        O = work.tile([P, P], F32, tag="O")
                        nc.vector.tensor_tensor(
                            out=O[:], in0=dlt[:, gt:gt + 1].to_broadcast([P, P]),
                            in1=iota_row[:], op=ALU.is_equal)
                        OT = work.tile([P, P], F32, tag="OT")
                        transpose_to_sbuf(O[:], OT[:])
                        adp = psum.tile([P, 4], F32, tag="ep")
                        nc.tensor.matmul(adp[:], lhsT=OT[:], rhs=ad_w[:, w, :],
                                         start=True, stop=True)
                        att = work.tile([P, 4], F32, tag="att")
                        nc.vector.tensor_add(out=att[:], in0=G[:, 512:516],
                                             in1=adp[:])
                        nc.vector.scalar_tensor_tensor(
                            out=att[:], in0=att[:], scalar=0.2, in1=att[:],
                            op0=ALU.mult, op1=ALU.max)
                        wt = work.tile([P, 4], F32, tag="wt")
                        nc.scalar.activation(wt[:], att[:], AF.Exp, bias=shift_t[:])
                        msg = work.tile([P, 4, 128], F32, tag="msg")
                        nc.vector.tensor_mul(
                            out=msg[:],
                            in0=G[:, 0:512].rearrange("p (h c) -> p h c", h=4),
                            in1=wt[:, :, None].to_broadcast([P, 4, 128]))
                        nc.tensor.matmul(Mw[:], lhsT=O[:],
                                         rhs=msg[:].rearrange("p h c -> p (h c)"),
                                         start=(t == 0), stop=(t == T - 1),
                                         skip_group_check=True)
                        nc.tensor.matmul(Dw[:], lhsT=O[:], rhs=wt[:],
                                         start=(t == 0), stop=(t == T - 1),
                                         skip_group_check=True)
                    # finalize: reload own h rows from cc_in, add self loop
                    rows = P if w < nw - 1 else nvalid
                    hre = work.tile([P, 512], F32, tag="hre")
                    if rows < P:
                        nc.vector.memset(hre[:], 0.0)
                    nc.sync.dma_start(out=hre[:rows, :],
                                      in_=cc_in[w * P:w * P + rows, 0:512])
                    es = work.tile([P, 4], F32, tag="es")
                    nc.vector.tensor_add(out=es[:], in0=as_w[:, w, :],
                                         in1=ad_w[:, w, :])
                    nc.vector.scalar_tensor_tensor(
                        out=es[:], in0=es[:], scalar=0.2, in1=es[:],
                        op0=ALU.mult, op1=ALU.max)
                    ws = work.tile([P, 4], F32, tag="ws")
                    nc.scalar.activation(ws[:], es[:], AF.Exp, bias=shift_t[:])
                    num = work.tile([P, 4, 128], F32, tag="num")
                    nc.vector.tensor_mul(
                        out=num[:], in0=hre[:].rearrange("p (h c) -> p h c", h=4),
                        in1=ws[:, :, None].to_broadcast([P, 4, 128]))
                    nc.vector.tensor_add(
                        out=num[:], in0=num[:],
                        in1=Mw[:].rearrange("p (h c) -> p h c", h=4))
                    den = work.tile([P, 4], F32, tag="den")
                    nc.vector.tensor_add(out=den[:], in0=Dw[:], in1=ws[:])
                    rec = work.tile([P, 4], F32, tag="rec")
                    nc.vector.reciprocal(rec[:], den[:])
                    o5 = work.tile([P, 4, 128], F32, tag="o5")
                    nc.vector.tensor_mul(
                        out=o5[:], in0=num[:],
                        in1=rec[:, :, None].to_broadcast([P, 4, 128]))
                    if concat:
                        ob = work.tile([P, 512], F32, tag="ob")
                        nc.vector.tensor_add(
                            out=ob[:], in0=o5[:].rearrange("p h c -> p (h c)"),
                            in1=bias_t[:])
                        mn = work.tile([P, 512], F32, tag="mn")
                        nc.vector.tensor_scalar_min(mn[:], ob[:], 0.0)
                        nc.scalar.activation(mn[:], mn[:], AF.Exp)
                        nc.vector.tensor_scalar_add(mn[:], mn[:], -1.0)
                        fin = work.tile([P, 512], F32, tag="fin")
                        nc.vector.tensor_tensor(out=fin[:], in0=ob[:], in1=mn[:],
                                                op=ALU.max)
                    else:
                        hm = work.tile([P, 128], F32, tag="hm")
                        nc.vector.tensor_add(out=hm[:], in0=o5[:, 0, :],
                                             in1=o5[:, 1, :])
                        nc.vector.tensor_add(out=hm[:], in0=hm[:], in1=o5[:, 2, :])
                        nc.vector.tensor_add(out=hm[:], in0=hm[:], in1=o5[:, 3, :])
                        nc.vector.tensor_scalar_mul(hm[:], hm[:], 0.25)
                        nc.vector.tensor_add(out=hm[:], in0=hm[:], in1=bias_t[:])
                        mn = work.tile([P, 128], F32, tag="mn2")
                        nc.vector.tensor_scalar_min(mn[:], hm[:], 0.0)
                        nc.scalar.activation(mn[:], mn[:], AF.Exp)
                        nc.vector.tensor_scalar_add(mn[:], mn[:], -1.0)
                        fin = work.tile([P, 128], F32, tag="fin2")
                        nc.vector.tensor_tensor(out=fin[:], in0=hm[:], in1=mn[:],
                                                op=ALU.max)
                    out_writer(w, rows, fin)
                    t0 += T

            def g1_write(w, rows, fin):
                nc.sync.dma_start(out=g1_dram[w * P:w * P + rows, :],
                                  in_=fin[:rows, :])

            gat_layer(1, lambda w, fc: (x1_w if fc == 0 else x2_w)[:, w, :], 2,
                      wg1t, asr1t, adr1t, cc_ing1, cc_outg1, sh1t, True, bg1t,
                      g1_write)

            g2_w = resid.tile([P, nw, 128], F32, tag="xC")

            def g1_chunk(w, fc):
                # reload g1 window chunk from DRAM
                gw = work.tile([P, P], F32, tag="gw")
                rows = P if w < nw - 1 else nvalid
                if rows < P:
                    nc.vector.memset(gw[:], 0.0)
                nc.sync.dma_start(out=gw[:rows, :],
                                  in_=g1_dram[w * P:w * P + rows,
                                              fc * 128:(fc + 1) * 128])
                return gw[:]

            def g2_write(w, rows, fin):
                nc.vector.tensor_copy(out=g2_w[:, w, :], in_=fin[:])

            gat_layer(2, g1_chunk, 4, wg2t, asr2t, adr2t, cc_ing2, cc_outg2,
                      sh2t, False, bg2t, g2_write)

            # ================= classifier =================
            for w in range(nw):
                fmt = work.tile([P, P], F32, tag="fm")
                transpose_to_sbuf(g2_w[:, w, :], fmt[:])
                po = psum.tile([P, 40], F32, tag="ep")
                nc.tensor.matmul(po[:], lhsT=fmt[:], rhs=wclst[:], start=True,
                                 stop=True)
                ot = work.tile([P, 40], F32, tag="ot")
                nc.vector.tensor_add(out=ot[:], in0=po[:], in1=bclst[:])
                rows = P if w < nw - 1 else nvalid
                nc.sync.dma_start(out=out_t[w * P:w * P + rows, :], in_=ot[:rows, :])

    split_sync_waits(nc)
    return nc


# ------------------------------------------------------------------ kernel

def make_in_maps(x, edge_index, params, g, n_cores):
    f1 = fold_ec(params["ec1"])
    f2 = fold_ec(params["ec2"])
    gp1, gp2 = params["gat1"], params["gat2"]
    n_own = g["n_own"]
    common = {}
    for l, f in ((1, f1), (2, f2)):
        for n in ("WA", "WB", "WSa", "WSb", "W2p", "W3"):
            common[f"ec{l}_{n}"] = _f32(f[n])
        common[f"ec{l}_c1r"] = rep(f["c1"])
        common[f"ec{l}_c2r"] = rep(f["c2"])
        common[f"ec{l}_cSr"] = rep(f["cS"])
    common["wg1"] = _f32(gp1["w"])
    common["wg2"] = _f32(gp2["w"])
    common["asrc1r"] = rep(_f32(gp1["a_src"]).reshape(-1))
    common["adst1r"] = rep(_f32(gp1["a_dst"]).reshape(-1))
    common["asrc2r"] = rep(_f32(gp2["a_src"]).reshape(-1))
    common["adst2r"] = rep(_f32(gp2["a_dst"]).reshape(-1))
    common["bg1r"] = rep(_f32(gp1["bias"]))
    common["bg2r"] = rep(_f32(gp2["bias"]))
    common["wcls"] = _f32(params["cls"]["w"])
    common["bclsr"] = rep(_f32(params["cls"]["b"]))
    in_maps = []
    for c in range(n_cores):
        m = dict(common)
        m["x_own"] = np.ascontiguousarray(x[c * n_own:(c + 1) * n_own])
        m["src_idx"] = np.ascontiguousarray(g["src_idx"][c])
        m["dstloc"] = np.ascontiguousarray(g["dstloc"][c])
        m["degw"] = np.ascontiguousarray(g["degw"][c])
        m["invw"] = np.ascontiguousarray(g["invw"][c])
        in_maps.append(m)
    return in_maps


def kernel(x, edge_index, params):
    x = np.ascontiguousarray(np.asarray(x, dtype=np.float32))
    n_cores = 8
    g = build_graph(edge_index, x.shape[0], n_cores)
    nc = build_program(g, n_cores)
    in_maps = make_in_maps(x, edge_index, params, g, n_cores)
    from concourse.bass_utils import run_bass_kernel_spmd
    res = run_bass_kernel_spmd(nc, in_maps, core_ids=list(range(n_cores)))
    return np.concatenate([res.results[c]["out"] for c in range(n_cores)], axis=0)
